# revision 7
# baseline (speedup 1.0000x reference)
"""Trainium2 Bass kernel for the single-step 2D wave-equation stencil
(nn_CustomRNN_12987981103489).

The reference computes, for the 2112x2112 interior of a 2128x2128 grid:

    out = 2*p2 - p1 + alpha * ((d2p/dx2) + (d2p/dy2))
    alpha = edge_pad(varray, 40)^2 * DT^2,  second differences / DX^2

plus a point-source add and an 8-wide zero frame. The Laplacian term is
scaled by varray^2 * DT^2 / DX^2 ~ 2.5e-9, i.e. it contributes ~1e-8
relative to the dominant f32 term 2*p2 - p1, so it is computed in bf16
on the TensorEngine (tridiagonal matmuls handle the partition-axis
stencil shifts, which the per-lane DVE cannot do), while the dominant
term stays in f32 on the VectorEngine. Measured vs the reference this
lands at ~2e-11 norm relative error.

Sharding: 1D row partition of the 2112 interior rows across 8 cores
(264 rows each); each core's p2 slice carries a 1-row halo on each side
so no inter-core communication is needed for this single step.

Layout trick: compute-engine APs must start at partition 0/32/64/96, so
each row tile keeps its 126 center rows at partitions 0..125 and stashes
the north/south halo rows at partitions 126/127; the halo contribution
to the vertical stencil is folded into the weight matrix columns.
"""

import json

import numpy as np
import ml_dtypes

import concourse.bass as bass
import concourse.mybir as mybir
from concourse.tile import TileContext
from concourse.vector_clock import ScopedClock
from concourse.bass_utils import run_bass_kernel_spmd

PML = 40
DT = 0.0005
DX = 10.0
NP = 2128          # padded grid size
INT = 2112         # interior size (8-wide frame)
NCORES = 8
ROWS = INT // NCORES   # 264 output rows per core
PCOLS = INT + 2        # 2114 p2 columns per core (1-col halo each side)
# per-core row tiles: (row offset within shard, output rows)
TILES = [(0, 126), (126, 126), (252, 12)]
COL_CHUNKS = [(0, 512), (512, 512), (1024, 512), (1536, 512), (2048, 64)]
F32 = mybir.dt.float32
BF16 = mybir.dt.bfloat16


def _install_ntff_hook():
    """Best-effort registration of the axon NTFF profile hook so
    trace=True (or BASS_TRACE=1) yields a real neuron-profile exec time.
    The image's antenv package lacks axon_hooks; provide the module shim
    and wire the ctypes hook from trn_agent_boot. Harmless if absent."""
    import sys as _sys
    import types as _types

    if "antenv.axon_hooks" in _sys.modules:
        return
    try:
        mod = _types.ModuleType("antenv.axon_hooks")
        state = {"hook": None}
        mod.set_axon_ntff_profile_hook = lambda h: state.__setitem__("hook", h)
        mod.get_axon_ntff_profile_hook = lambda: state["hook"]
        from trn_agent_boot.trn_boot import _ntff_profile_via_ctypes

        hook = _ntff_profile_via_ctypes("/opt/axon/libaxon_pjrt.so")
        mod.set_axon_ntff_profile_hook(hook)
        _sys.modules["antenv.axon_hooks"] = mod
    except Exception:
        pass


class _SplitDrainTileContext(TileContext):
    """The walrus build in this container rejects >2 sync waits on one
    TPB_CTRL instruction; the stock TileContext funnels the whole global
    clock into a single kernel-tail Drain. Re-emit those waits one per
    SP NOP instead."""

    def _drain_and_barrier(self, tick_clock, wait_clock):
        drain_inst = self.nc.sync.drain()
        wait_clock.add_sem_waits(
            drain_inst.ins, ScopedClock({None: tick_clock.global_clock})
        )
        si = drain_inst.ins.sync_info
        waits = list(si.on_wait) if si is not None else []
        if len(waits) > 1:
            drain_inst.ins.sync_info = mybir.SyncInfo(
                on_update=list(si.on_update), on_wait=[]
            )
            for w in waits:
                nop = self.nc.sync.nop(nofuse=True, hint="split_drain_wait")
                nop.ins.sync_info = mybir.SyncInfo(on_update=[], on_wait=[w])

        self.nc.all_engine_barrier()
        assert self.sems is not None
        popped = self.nc._tile_sem_poison_stack.pop()
        assert popped is self._sem_poison
        self.nc.clear_and_free_semaphores(list(self.sems.allocated().values()))
        self.nc.all_engine_barrier()


def _lap_weights(p):
    """Vertical-stencil weight matrix for a tile with p center rows at
    partitions 0..p-1, north halo row at partition p, south halo row at
    partition p+1 (lhsT layout: out = lhsT.T @ rhs, contraction over
    partitions). Out row j = -4*C[j] + C[j-1] + C[j+1] (+halo at the
    edges); columns >= p are zero so the unused psum rows read 0."""
    w = np.zeros((128, 128), np.float32)
    for j in range(p):
        w[j, j] = -4.0
        if j > 0:
            w[j - 1, j] = 1.0
        if j < p - 1:
            w[j + 1, j] = 1.0
    w[p, 0] = 1.0        # north halo feeds out row 0
    w[p + 1, p - 1] = 1.0  # south halo feeds out row p-1
    return w.astype(ml_dtypes.bfloat16)


def _build_weights():
    wlap = _lap_weights(126)
    wtail = _lap_weights(12)
    wid = np.eye(128, dtype=np.float32).astype(ml_dtypes.bfloat16)
    return wlap, wtail, wid


def _build_nc():
    nc = bass.Bass()
    p1s = nc.dram_tensor("p1s", [ROWS, INT], F32, kind="ExternalInput")
    p2s = nc.dram_tensor("p2s", [ROWS + 2, PCOLS], F32, kind="ExternalInput")
    vas = nc.dram_tensor("vas", [ROWS, INT], F32, kind="ExternalInput")
    wlap = nc.dram_tensor("wlap", [128, 128], BF16, kind="ExternalInput")
    wtail = nc.dram_tensor("wtail", [128, 128], BF16, kind="ExternalInput")
    wid = nc.dram_tensor("wid", [128, 128], BF16, kind="ExternalInput")
    outs = nc.dram_tensor("outs", [ROWS, INT], F32, kind="ExternalOutput")

    with _SplitDrainTileContext(nc) as tc:
        with (
            tc.tile_pool(name="wpool", bufs=1) as wpool,
            tc.tile_pool(name="sbuf", bufs=3) as pool,
            tc.tile_pool(name="psum", bufs=1, space="PSUM") as ppool,
        ):
            wlap_t = wpool.tile([128, 128], BF16, tag="wlap")
            nc.gpsimd.dma_start(out=wlap_t[:], in_=wlap[:])
            wtail_t = wpool.tile([128, 128], BF16, tag="wtail")
            nc.gpsimd.dma_start(out=wtail_t[:], in_=wtail[:])
            wid_t = wpool.tile([128, 128], BF16, tag="wid")
            nc.gpsimd.dma_start(out=wid_t[:], in_=wid[:])

            for r0, p in TILES:
                k = p + 2  # center rows + 2 halo rows
                wv = wlap_t if p == 126 else wtail_t
                # DMA issue is spread over SP (p2) and POOL/SWDGE (p1,
                # varray, out) — a single sequencer serializes at ~1us
                # of descriptor-enqueue time per big DMA.
                vat = pool.tile([128, INT], F32, tag="vat")
                nc.gpsimd.dma_start(out=vat[:p, :], in_=vas[r0 : r0 + p, :])
                p2t = pool.tile([128, PCOLS], F32, tag="p2t")
                # center rows r0..r0+p-1 (p2s local rows r0+1..r0+p)
                nc.sync.dma_start(
                    out=p2t[:p, :], in_=p2s[r0 + 1 : r0 + 1 + p, :]
                )
                # halo rows r0-1 and r0+p (p2s local r0 and r0+p+1)
                nc.sync.dma_start(
                    out=p2t[p : p + 2, :],
                    in_=p2s[r0 : r0 + p + 2 : p + 1, :],
                )
                p1t = pool.tile([128, INT], F32, tag="p1t")
                nc.gpsimd.dma_start(out=p1t[:p, :], in_=p1s[r0 : r0 + p, :])

                # bf16 copy of p2 for the TensorEngine stencil
                p2b = pool.tile([128, PCOLS], BF16, tag="p2b")
                nc.scalar.copy(out=p2b[:k, :], in_=p2t[:k, :])
                # alpha/dx^2 = (varray * (DT/DX))^2
                ac = pool.tile([128, INT], F32, tag="ac")
                nc.scalar.activation(
                    out=ac[:p, :],
                    in_=vat[:p, :],
                    func=mybir.ActivationFunctionType.Square,
                    scale=float(DT / DX),
                )

                # 5-point Laplacian accumulated in PSUM: vertical part
                # incl. halo via wv, then +west, +east (grouped by weight
                # matrix to minimize LDWEIGHTS swaps)
                psum = ppool.tile([128, INT], F32, tag="psum")
                for c0, w in COL_CHUNKS:
                    nc.tensor.matmul(
                        psum[:, c0 : c0 + w],
                        wv[:k, :],
                        p2b[:k, 1 + c0 : 1 + c0 + w],
                        start=True,
                        stop=False,
                    )
                for c0, w in COL_CHUNKS:
                    nc.tensor.matmul(
                        psum[:, c0 : c0 + w],
                        wid_t[:p, :],
                        p2b[:p, c0 : c0 + w],
                        start=False,
                        stop=False,
                    )
                for c0, w in COL_CHUNKS:
                    nc.tensor.matmul(
                        psum[:, c0 : c0 + w],
                        wid_t[:p, :],
                        p2b[:p, 2 + c0 : 2 + c0 + w],
                        start=False,
                        stop=True,
                    )

                prod = pool.tile([128, INT], F32, tag="prod")
                nc.vector.tensor_mul(prod[:p, :], psum[:p, :], ac[:p, :])
                tt = pool.tile([128, INT], F32, tag="tt")
                nc.vector.scalar_tensor_tensor(
                    out=tt[:p, :],
                    in0=p2t[:p, 1 : 1 + INT],
                    scalar=2.0,
                    in1=p1t[:p, :],
                    op0=mybir.AluOpType.mult,
                    op1=mybir.AluOpType.subtract,
                )
                # final sum lands in vat (dead after ac) to save a slot
                nc.vector.tensor_add(vat[:p, :], tt[:p, :], prod[:p, :])
                nc.gpsimd.dma_start(out=outs[r0 : r0 + p, :], in_=vat[:p, :])
    return nc


_ENGINES = {"SP", "PE", "DVE", "Activation", "Pool"}


def _split_waits_json(raw, maxw=1):
    """The walrus build here rejects instructions carrying more than a
    couple of semaphore waits. Engine streams execute in order, so any
    excess waits can be hoisted onto same-engine NoOps emitted directly
    before the instruction — semantically identical, codegen-legal."""
    m = json.loads(raw)
    n = 0
    for f in m["functions"]:
        for blk in f["blocks"]:
            out = []
            for inst in blk["instructions"]:
                si = inst.get("sync_info")
                waits = (si or {}).get("on_wait") or []
                if len(waits) > maxw and inst.get("engine") in _ENGINES:
                    for w in waits[:-maxw]:
                        n += 1
                        out.append(
                            {
                                "name": f"I-splitw{n}",
                                "opcode": "NoOp",
                                "engine": inst["engine"],
                                "ins": [],
                                "outs": [],
                                "sync_info": {"on_update": [], "on_wait": [w]},
                            }
                        )
                    si["on_wait"] = waits[-maxw:]
                out.append(inst)
            blk["instructions"] = out
    return json.dumps(m).encode()


_CACHE = {}


def _get_nc():
    if "nc" not in _CACHE:
        nc = _build_nc()
        orig = nc.to_json_bytes
        nc.to_json_bytes = lambda: _split_waits_json(orig())
        _CACHE["nc"] = nc
    return _CACHE["nc"]


def _shard_inputs(p1, p2, varray):
    p1 = np.asarray(p1, np.float32)
    p2 = np.asarray(p2, np.float32)
    varray = np.asarray(varray, np.float32)
    wlap, wtail, wid = _CACHE.setdefault("weights", _build_weights())
    ci = np.clip(np.arange(8, 8 + INT) - PML, 0, varray.shape[1] - 1)
    in_maps = []
    for c in range(NCORES):
        g0 = 8 + ROWS * c
        ri = np.clip(np.arange(g0, g0 + ROWS) - PML, 0, varray.shape[0] - 1)
        in_maps.append(
            {
                "p1s": np.ascontiguousarray(p1[g0 : g0 + ROWS, 8 : 8 + INT]),
                "p2s": np.ascontiguousarray(
                    p2[g0 - 1 : g0 + ROWS + 1, 7 : 7 + PCOLS]
                ),
                "vas": np.ascontiguousarray(varray[np.ix_(ri, ci)]),
                "wlap": wlap,
                "wtail": wtail,
                "wid": wid,
            }
        )
    return in_maps


def run(p1, p2, varray, source_function, x_s, y_s, t, trace=False):
    """Run the device kernel; returns ((p, col), BassKernelResults)."""
    if trace:
        _install_ntff_hook()
    in_maps = _shard_inputs(p1, p2, varray)
    res = run_bass_kernel_spmd(
        _get_nc(), in_maps, core_ids=list(range(NCORES)), trace=trace
    )
    p = np.zeros((NP, NP), np.float32)
    for c in range(NCORES):
        g0 = 8 + ROWS * c
        p[g0 : g0 + ROWS, 8 : 8 + INT] = res.results[c]["outs"]
    sf = np.asarray(source_function)
    p[int(x_s) + PML, int(y_s) + PML] += np.float32(float(sf[int(t)]) * DT**2)
    col = np.ascontiguousarray(p[PML:-PML, PML:-PML][:, 50])
    return (p, col), res


def kernel(p1, p2, varray, source_function, x_s, y_s, t):
    (p, col), _ = run(p1, p2, varray, source_function, x_s, y_s, t)
    return (p, col)


# revision 8
# speedup vs baseline: 1.2880x; 1.2880x over previous
"""Trainium2 Bass kernel for the single-step 2D wave-equation stencil
(nn_CustomRNN_12987981103489).

The reference computes, for the 2112x2112 interior of a 2128x2128 grid:

    out = 2*p2 - p1 + alpha * ((d2p/dx2) + (d2p/dy2))
    alpha = edge_pad(varray, 40)^2 * DT^2,  second differences / DX^2

plus a point-source add and an 8-wide zero frame. The Laplacian term is
scaled by varray^2 * DT^2 / DX^2 ~ 2.5e-9, i.e. it contributes ~1e-8
relative to the dominant f32 term 2*p2 - p1, so it is computed in bf16
on the TensorEngine (tridiagonal matmuls handle the partition-axis
stencil shifts, which the per-lane DVE cannot do), while the dominant
term stays in f32 on the VectorEngine. Measured vs the reference this
lands at ~2e-11 norm relative error.

Sharding: 1D row partition of the 2112 interior rows across 8 cores
(264 rows each); each core's p2 slice carries a 1-row halo on each side
so no inter-core communication is needed for this single step.

Layout trick: compute-engine APs must start at partition 0/32/64/96, so
each row tile keeps its 126 center rows at partitions 0..125 and stashes
the north/south halo rows at partitions 126/127; the halo contribution
to the vertical stencil is folded into the weight matrix columns.
"""

import json

import numpy as np
import ml_dtypes

import concourse.bass as bass
import concourse.mybir as mybir
from concourse.tile import TileContext
from concourse.vector_clock import ScopedClock
from concourse.bass_utils import run_bass_kernel_spmd

PML = 40
DT = 0.0005
DX = 10.0
NP = 2128          # padded grid size
INT = 2112         # interior size (8-wide frame)
NCORES = 8
ROWS = INT // NCORES   # 264 output rows per core
PCOLS = INT + 2        # 2114 p2 columns per core (1-col halo each side)
# per-core row tiles: (row offset within shard, output rows)
TILES = [(0, 126), (126, 126), (252, 12)]
COL_CHUNKS = [(0, 512), (512, 512), (1024, 512), (1536, 512), (2048, 64)]
F32 = mybir.dt.float32
BF16 = mybir.dt.bfloat16


def _install_ntff_hook():
    """Best-effort registration of the axon NTFF profile hook so
    trace=True (or BASS_TRACE=1) yields a real neuron-profile exec time.
    The image's antenv package lacks axon_hooks; provide the module shim
    and wire the ctypes hook from trn_agent_boot. Harmless if absent."""
    import sys as _sys
    import types as _types

    if "antenv.axon_hooks" in _sys.modules:
        return
    try:
        mod = _types.ModuleType("antenv.axon_hooks")
        state = {"hook": None}
        mod.set_axon_ntff_profile_hook = lambda h: state.__setitem__("hook", h)
        mod.get_axon_ntff_profile_hook = lambda: state["hook"]
        from trn_agent_boot.trn_boot import _ntff_profile_via_ctypes

        hook = _ntff_profile_via_ctypes("/opt/axon/libaxon_pjrt.so")
        mod.set_axon_ntff_profile_hook(hook)
        _sys.modules["antenv.axon_hooks"] = mod
    except Exception:
        pass


class _SplitDrainTileContext(TileContext):
    """The walrus build in this container rejects >2 sync waits on one
    TPB_CTRL instruction; the stock TileContext funnels the whole global
    clock into a single kernel-tail Drain. Re-emit those waits one per
    SP NOP instead."""

    def _drain_and_barrier(self, tick_clock, wait_clock):
        drain_inst = self.nc.sync.drain()
        wait_clock.add_sem_waits(
            drain_inst.ins, ScopedClock({None: tick_clock.global_clock})
        )
        si = drain_inst.ins.sync_info
        waits = list(si.on_wait) if si is not None else []
        if len(waits) > 1:
            drain_inst.ins.sync_info = mybir.SyncInfo(
                on_update=list(si.on_update), on_wait=[]
            )
            for w in waits:
                nop = self.nc.sync.nop(nofuse=True, hint="split_drain_wait")
                nop.ins.sync_info = mybir.SyncInfo(on_update=[], on_wait=[w])

        self.nc.all_engine_barrier()
        assert self.sems is not None
        popped = self.nc._tile_sem_poison_stack.pop()
        assert popped is self._sem_poison
        self.nc.clear_and_free_semaphores(list(self.sems.allocated().values()))
        self.nc.all_engine_barrier()


def _lap_weights(p):
    """Vertical-stencil weight matrix for a tile with p center rows at
    partitions 0..p-1, north halo row at partition p, south halo row at
    partition p+1 (lhsT layout: out = lhsT.T @ rhs, contraction over
    partitions). Out row j = -4*C[j] + C[j-1] + C[j+1] (+halo at the
    edges); columns >= p are zero so the unused psum rows read 0."""
    w = np.zeros((128, 128), np.float32)
    for j in range(p):
        w[j, j] = -4.0
        if j > 0:
            w[j - 1, j] = 1.0
        if j < p - 1:
            w[j + 1, j] = 1.0
    w[p, 0] = 1.0        # north halo feeds out row 0
    w[p + 1, p - 1] = 1.0  # south halo feeds out row p-1
    return w.astype(ml_dtypes.bfloat16)


def _build_weights():
    wlap = _lap_weights(126)
    wtail = _lap_weights(12)
    wid = np.eye(128, dtype=np.float32).astype(ml_dtypes.bfloat16)
    return wlap, wtail, wid


def _build_nc():
    nc = bass.Bass()
    p1s = nc.dram_tensor("p1s", [ROWS, INT], F32, kind="ExternalInput")
    p2s = nc.dram_tensor("p2s", [ROWS + 2, PCOLS], F32, kind="ExternalInput")
    vas = nc.dram_tensor("vas", [ROWS, INT], F32, kind="ExternalInput")
    wlap = nc.dram_tensor("wlap", [128, 128], BF16, kind="ExternalInput")
    wtail = nc.dram_tensor("wtail", [128, 128], BF16, kind="ExternalInput")
    wid = nc.dram_tensor("wid", [128, 128], BF16, kind="ExternalInput")
    outs = nc.dram_tensor("outs", [ROWS, INT], F32, kind="ExternalOutput")

    with _SplitDrainTileContext(nc) as tc:
        with (
            tc.tile_pool(name="wpool", bufs=1) as wpool,
            tc.tile_pool(name="sbuf", bufs=3) as pool,
            tc.tile_pool(name="psum", bufs=1, space="PSUM") as ppool,
        ):
            wlap_t = wpool.tile([128, 128], BF16, tag="wlap")
            nc.sync.dma_start(out=wlap_t[:], in_=wlap[:])
            wtail_t = wpool.tile([128, 128], BF16, tag="wtail")
            nc.sync.dma_start(out=wtail_t[:], in_=wtail[:])
            wid_t = wpool.tile([128, 128], BF16, tag="wid")
            nc.sync.dma_start(out=wid_t[:], in_=wid[:])

            for r0, p in TILES:
                k = p + 2  # center rows + 2 halo rows
                wv = wlap_t if p == 126 else wtail_t
                # DMA issue is spread over the two HWDGE engines (SP: p2
                # + out, ACT: p1 + varray) — a single sequencer
                # serializes at ~1us of descriptor-enqueue per big DMA,
                # and SWDGE (gpsimd) queues move bulk data at only
                # ~6 GB/s, so everything bulk stays on HWDGE.
                vat = pool.tile([128, INT], F32, tag="vat")
                nc.scalar.dma_start(out=vat[:p, :], in_=vas[r0 : r0 + p, :])
                p2t = pool.tile([128, PCOLS], F32, tag="p2t")
                # center rows r0..r0+p-1 (p2s local rows r0+1..r0+p)
                nc.sync.dma_start(
                    out=p2t[:p, :], in_=p2s[r0 + 1 : r0 + 1 + p, :]
                )
                # halo rows r0-1 and r0+p (p2s local r0 and r0+p+1)
                nc.sync.dma_start(
                    out=p2t[p : p + 2, :],
                    in_=p2s[r0 : r0 + p + 2 : p + 1, :],
                )
                p1t = pool.tile([128, INT], F32, tag="p1t")
                nc.scalar.dma_start(out=p1t[:p, :], in_=p1s[r0 : r0 + p, :])

                # bf16 copy of p2 for the TensorEngine stencil
                p2b = pool.tile([128, PCOLS], BF16, tag="p2b")
                nc.scalar.copy(out=p2b[:k, :], in_=p2t[:k, :])
                # alpha/dx^2 = (varray * (DT/DX))^2
                ac = pool.tile([128, INT], F32, tag="ac")
                nc.scalar.activation(
                    out=ac[:p, :],
                    in_=vat[:p, :],
                    func=mybir.ActivationFunctionType.Square,
                    scale=float(DT / DX),
                )

                # 5-point Laplacian accumulated in PSUM: vertical part
                # incl. halo via wv, then +west, +east (grouped by weight
                # matrix to minimize LDWEIGHTS swaps)
                psum = ppool.tile([128, INT], F32, tag="psum")
                for c0, w in COL_CHUNKS:
                    nc.tensor.matmul(
                        psum[:, c0 : c0 + w],
                        wv[:k, :],
                        p2b[:k, 1 + c0 : 1 + c0 + w],
                        start=True,
                        stop=False,
                    )
                for c0, w in COL_CHUNKS:
                    nc.tensor.matmul(
                        psum[:, c0 : c0 + w],
                        wid_t[:p, :],
                        p2b[:p, c0 : c0 + w],
                        start=False,
                        stop=False,
                    )
                for c0, w in COL_CHUNKS:
                    nc.tensor.matmul(
                        psum[:, c0 : c0 + w],
                        wid_t[:p, :],
                        p2b[:p, 2 + c0 : 2 + c0 + w],
                        start=False,
                        stop=True,
                    )

                prod = pool.tile([128, INT], F32, tag="prod")
                nc.vector.tensor_mul(prod[:p, :], psum[:p, :], ac[:p, :])
                tt = pool.tile([128, INT], F32, tag="tt")
                nc.vector.scalar_tensor_tensor(
                    out=tt[:p, :],
                    in0=p2t[:p, 1 : 1 + INT],
                    scalar=2.0,
                    in1=p1t[:p, :],
                    op0=mybir.AluOpType.mult,
                    op1=mybir.AluOpType.subtract,
                )
                # final sum lands in vat (dead after ac) to save a slot
                nc.vector.tensor_add(vat[:p, :], tt[:p, :], prod[:p, :])
                nc.sync.dma_start(out=outs[r0 : r0 + p, :], in_=vat[:p, :])
    return nc


_ENGINES = {"SP", "PE", "DVE", "Activation", "Pool"}


def _split_waits_json(raw, maxw=1):
    """The walrus build here rejects instructions carrying more than a
    couple of semaphore waits. Engine streams execute in order, so any
    excess waits can be hoisted onto same-engine NoOps emitted directly
    before the instruction — semantically identical, codegen-legal."""
    m = json.loads(raw)
    n = 0
    for f in m["functions"]:
        for blk in f["blocks"]:
            out = []
            for inst in blk["instructions"]:
                si = inst.get("sync_info")
                waits = (si or {}).get("on_wait") or []
                if len(waits) > maxw and inst.get("engine") in _ENGINES:
                    for w in waits[:-maxw]:
                        n += 1
                        out.append(
                            {
                                "name": f"I-splitw{n}",
                                "opcode": "NoOp",
                                "engine": inst["engine"],
                                "ins": [],
                                "outs": [],
                                "sync_info": {"on_update": [], "on_wait": [w]},
                            }
                        )
                    si["on_wait"] = waits[-maxw:]
                out.append(inst)
            blk["instructions"] = out
    return json.dumps(m).encode()


_CACHE = {}


def _get_nc():
    if "nc" not in _CACHE:
        nc = _build_nc()
        orig = nc.to_json_bytes
        nc.to_json_bytes = lambda: _split_waits_json(orig())
        _CACHE["nc"] = nc
    return _CACHE["nc"]


def _shard_inputs(p1, p2, varray):
    p1 = np.asarray(p1, np.float32)
    p2 = np.asarray(p2, np.float32)
    varray = np.asarray(varray, np.float32)
    wlap, wtail, wid = _CACHE.setdefault("weights", _build_weights())
    ci = np.clip(np.arange(8, 8 + INT) - PML, 0, varray.shape[1] - 1)
    in_maps = []
    for c in range(NCORES):
        g0 = 8 + ROWS * c
        ri = np.clip(np.arange(g0, g0 + ROWS) - PML, 0, varray.shape[0] - 1)
        in_maps.append(
            {
                "p1s": np.ascontiguousarray(p1[g0 : g0 + ROWS, 8 : 8 + INT]),
                "p2s": np.ascontiguousarray(
                    p2[g0 - 1 : g0 + ROWS + 1, 7 : 7 + PCOLS]
                ),
                "vas": np.ascontiguousarray(varray[np.ix_(ri, ci)]),
                "wlap": wlap,
                "wtail": wtail,
                "wid": wid,
            }
        )
    return in_maps


def run(p1, p2, varray, source_function, x_s, y_s, t, trace=False):
    """Run the device kernel; returns ((p, col), BassKernelResults)."""
    if trace:
        _install_ntff_hook()
    in_maps = _shard_inputs(p1, p2, varray)
    res = run_bass_kernel_spmd(
        _get_nc(), in_maps, core_ids=list(range(NCORES)), trace=trace
    )
    p = np.zeros((NP, NP), np.float32)
    for c in range(NCORES):
        g0 = 8 + ROWS * c
        p[g0 : g0 + ROWS, 8 : 8 + INT] = res.results[c]["outs"]
    sf = np.asarray(source_function)
    p[int(x_s) + PML, int(y_s) + PML] += np.float32(float(sf[int(t)]) * DT**2)
    col = np.ascontiguousarray(p[PML:-PML, PML:-PML][:, 50])
    return (p, col), res


def kernel(p1, p2, varray, source_function, x_s, y_s, t):
    (p, col), _ = run(p1, p2, varray, source_function, x_s, y_s, t)
    return (p, col)


# revision 14
# speedup vs baseline: 1.2968x; 1.0068x over previous
"""Trainium2 Bass kernel for the single-step 2D wave-equation stencil
(nn_CustomRNN_12987981103489).

The reference computes, for the 2112x2112 interior of a 2128x2128 grid:

    out = 2*p2 - p1 + alpha * ((d2p/dx2) + (d2p/dy2))
    alpha = edge_pad(varray, 40)^2 * DT^2,  second differences / DX^2

plus a point-source add and an 8-wide zero frame. The Laplacian term is
scaled by varray^2 * DT^2 / DX^2 ~ 2.5e-9, i.e. it contributes ~1e-8
relative to the dominant f32 term 2*p2 - p1, so it is computed in bf16
on the TensorEngine (tridiagonal matmuls handle the partition-axis
stencil shifts, which the per-lane DVE cannot do), while the dominant
term stays in f32 on the VectorEngine. Measured vs the reference this
lands at ~2e-11 norm relative error.

Sharding: 1D row partition of the 2112 interior rows across 8 cores
(264 rows each); each core's p2 slice carries a 1-row halo on each side
so no inter-core communication is needed for this single step.

Layout trick: compute-engine APs must start at partition 0/32/64/96, so
each row tile keeps its 126 center rows at partitions 0..125 and stashes
the north/south halo rows at partitions 126/127; the halo contribution
to the vertical stencil is folded into the weight matrix columns.
"""

import json

import numpy as np
import ml_dtypes

import concourse.bass as bass
import concourse.mybir as mybir
from concourse.tile import TileContext
from concourse.vector_clock import ScopedClock
from concourse.bass_utils import run_bass_kernel_spmd

PML = 40
DT = 0.0005
DX = 10.0
NP = 2128          # padded grid size
INT = 2112         # interior size (8-wide frame)
NCORES = 8
ROWS = INT // NCORES   # 264 output rows per core
PCOLS = INT + 2        # 2114 p2 columns per core (1-col halo each side)
# per-core full row tiles: (row offset within shard, output rows); the
# 12-row remainder is handled separately in a column-stacked layout
TILES = [(0, 126), (126, 126)]
COL_CHUNKS = [(0, 512), (512, 512), (1024, 512), (1536, 512), (2048, 64)]
# tail: rows 252..263 restacked as 8 column-chunks of 264 cols
TR0 = 252          # first tail row (shard coords)
TROWS = 12
TG = 8             # column chunks
TCW = INT // TG    # 264 columns per chunk
F32 = mybir.dt.float32
BF16 = mybir.dt.bfloat16


def _install_ntff_hook():
    """Best-effort registration of the axon NTFF profile hook so
    trace=True (or BASS_TRACE=1) yields a real neuron-profile exec time.
    The image's antenv package lacks axon_hooks; provide the module shim
    and wire the ctypes hook from trn_agent_boot. Harmless if absent."""
    import sys as _sys
    import types as _types

    if "antenv.axon_hooks" in _sys.modules:
        return
    try:
        mod = _types.ModuleType("antenv.axon_hooks")
        state = {"hook": None}
        mod.set_axon_ntff_profile_hook = lambda h: state.__setitem__("hook", h)
        mod.get_axon_ntff_profile_hook = lambda: state["hook"]
        from trn_agent_boot.trn_boot import _ntff_profile_via_ctypes

        hook = _ntff_profile_via_ctypes("/opt/axon/libaxon_pjrt.so")
        mod.set_axon_ntff_profile_hook(hook)
        _sys.modules["antenv.axon_hooks"] = mod
    except Exception:
        pass


class _SplitDrainTileContext(TileContext):
    """The walrus build in this container rejects >2 sync waits on one
    TPB_CTRL instruction; the stock TileContext funnels the whole global
    clock into a single kernel-tail Drain. Re-emit those waits one per
    SP NOP instead."""

    def _drain_and_barrier(self, tick_clock, wait_clock):
        drain_inst = self.nc.sync.drain()
        wait_clock.add_sem_waits(
            drain_inst.ins, ScopedClock({None: tick_clock.global_clock})
        )
        si = drain_inst.ins.sync_info
        waits = list(si.on_wait) if si is not None else []
        if len(waits) > 1:
            drain_inst.ins.sync_info = mybir.SyncInfo(
                on_update=list(si.on_update), on_wait=[]
            )
            for w in waits:
                nop = self.nc.sync.nop(nofuse=True, hint="split_drain_wait")
                nop.ins.sync_info = mybir.SyncInfo(on_update=[], on_wait=[w])

        self.nc.all_engine_barrier()
        assert self.sems is not None
        popped = self.nc._tile_sem_poison_stack.pop()
        assert popped is self._sem_poison
        self.nc.clear_and_free_semaphores(list(self.sems.allocated().values()))
        self.nc.all_engine_barrier()


def _lap_weights(p):
    """Vertical-stencil weight matrix for a tile with p center rows at
    partitions 0..p-1, north halo row at partition p, south halo row at
    partition p+1 (lhsT layout: out = lhsT.T @ rhs, contraction over
    partitions). Out row j = -4*C[j] + C[j-1] + C[j+1] (+halo at the
    edges); columns >= p are zero so the unused psum rows read 0."""
    w = np.zeros((128, 128), np.float32)
    for j in range(p):
        w[j, j] = -4.0
        if j > 0:
            w[j - 1, j] = 1.0
        if j < p - 1:
            w[j + 1, j] = 1.0
    w[p, 0] = 1.0        # north halo feeds out row 0
    w[p + 1, p - 1] = 1.0  # south halo feeds out row p-1
    return w.astype(ml_dtypes.bfloat16)


def _tail_weights():
    """Block weights for the column-stacked 12-row tail: TG blocks, each
    14 input partitions (12 rows + both halos, natural order) and 12
    output partitions."""
    wtb = np.zeros((128, 128), np.float32)
    wti = np.zeros((128, 128), np.float32)
    for g in range(TG):
        bk, bj = 14 * g, TROWS * g
        for j in range(TROWS):
            wtb[bk + j, bj + j] = 1.0       # north
            wtb[bk + j + 1, bj + j] = -4.0  # center
            wtb[bk + j + 2, bj + j] = 1.0   # south
            wti[bk + j + 1, bj + j] = 1.0   # center identity (E/W)
    return wtb.astype(ml_dtypes.bfloat16), wti.astype(ml_dtypes.bfloat16)


def _build_weights():
    wlap = _lap_weights(126)
    wid = np.eye(128, dtype=np.float32).astype(ml_dtypes.bfloat16)
    wtb, wti = _tail_weights()
    return wlap, wid, wtb, wti


def _build_nc():
    nc = bass.Bass()
    p1s = nc.dram_tensor("p1s", [ROWS - TROWS, INT], F32, kind="ExternalInput")
    p2s = nc.dram_tensor("p2s", [ROWS + 2, PCOLS], F32, kind="ExternalInput")
    vas = nc.dram_tensor("vas", [ROWS - TROWS, INT], F32, kind="ExternalInput")
    wlap = nc.dram_tensor("wlap", [128, 128], BF16, kind="ExternalInput")
    wid = nc.dram_tensor("wid", [128, 128], BF16, kind="ExternalInput")
    wtb = nc.dram_tensor("wtb", [128, 128], BF16, kind="ExternalInput")
    wti = nc.dram_tensor("wti", [128, 128], BF16, kind="ExternalInput")
    # column-stacked 12-row tail (prepared host-side): TG blocks of
    # (12+2 halo) p2 rows / 12 rows for p1+varray, 264(+2) cols each
    p2tl = nc.dram_tensor("p2tl", [14 * TG, TCW + 2], F32, kind="ExternalInput")
    p2ctl = nc.dram_tensor("p2ctl", [TROWS * TG, TCW + 2], F32, kind="ExternalInput")
    p1tl = nc.dram_tensor("p1tl", [TROWS * TG, TCW], F32, kind="ExternalInput")
    vatl = nc.dram_tensor("vatl", [TROWS * TG, TCW], F32, kind="ExternalInput")
    outs = nc.dram_tensor("outs", [ROWS - TROWS, INT], F32, kind="ExternalOutput")
    outs_t = nc.dram_tensor(
        "outs_t", [TROWS * TG, TCW], F32, kind="ExternalOutput"
    )

    with _SplitDrainTileContext(nc) as tc:
        with (
            tc.tile_pool(name="wpool", bufs=1) as wpool,
            tc.tile_pool(name="sbuf", bufs=3) as pool,
            tc.tile_pool(name="tpool", bufs=1) as tpool,
            tc.tile_pool(name="psum", bufs=1, space="PSUM") as ppool,
        ):
            first = True
            for r0, p in TILES:
                k = p + 2  # center rows + 2 halo rows
                # DMA issue is spread over the two HWDGE engines (SP: p2
                # + out, ACT: p1 + varray) — a single sequencer
                # serializes at ~1us of descriptor-enqueue per big DMA,
                # and SWDGE (gpsimd) queues move bulk data at only
                # ~6 GB/s, so everything bulk stays on HWDGE.
                vat = pool.tile([128, INT], F32, tag="vat")
                nc.scalar.dma_start(out=vat[:p, :], in_=vas[r0 : r0 + p, :])
                p2t = pool.tile([128, PCOLS], F32, tag="p2t")
                # center rows r0..r0+p-1 (p2s local rows r0+1..r0+p)
                nc.sync.dma_start(
                    out=p2t[:p, :], in_=p2s[r0 + 1 : r0 + 1 + p, :]
                )
                # halo rows r0-1 and r0+p (p2s local r0 and r0+p+1)
                nc.sync.dma_start(
                    out=p2t[p : p + 2, :],
                    in_=p2s[r0 : r0 + p + 2 : p + 1, :],
                )
                p1t = pool.tile([128, INT], F32, tag="p1t")
                nc.scalar.dma_start(out=p1t[:p, :], in_=p1s[r0 : r0 + p, :])

                if first:
                    # weights issue after tile 0's bulk loads so they
                    # don't delay the pipeline-filling transfers
                    first = False
                    wlap_t = wpool.tile([128, 128], BF16, tag="wlap")
                    nc.sync.dma_start(out=wlap_t[:], in_=wlap[:])
                    wid_t = wpool.tile([128, 128], BF16, tag="wid")
                    nc.sync.dma_start(out=wid_t[:], in_=wid[:])
                    wtb_t = wpool.tile([128, 128], BF16, tag="wtb")
                    nc.scalar.dma_start(out=wtb_t[:], in_=wtb[:])
                    wti_t = wpool.tile([128, 128], BF16, tag="wti")
                    nc.scalar.dma_start(out=wti_t[:], in_=wti[:])
                wv = wlap_t

                # bf16 copy of p2 for the TensorEngine stencil
                p2b = pool.tile([128, PCOLS], BF16, tag="p2b")
                nc.scalar.copy(out=p2b[:k, :], in_=p2t[:k, :])
                # alpha/dx^2 = (varray * (DT/DX))^2
                ac = pool.tile([128, INT], F32, tag="ac")
                nc.scalar.activation(
                    out=ac[:p, :],
                    in_=vat[:p, :],
                    func=mybir.ActivationFunctionType.Square,
                    scale=float(DT / DX),
                )

                # 5-point Laplacian accumulated in PSUM: vertical part
                # incl. halo via wv, then +west, +east (grouped by weight
                # matrix to minimize LDWEIGHTS swaps)
                psum = ppool.tile([128, INT], F32, tag="psum")
                for c0, w in COL_CHUNKS:
                    nc.tensor.matmul(
                        psum[:, c0 : c0 + w],
                        wv[:k, :],
                        p2b[:k, 1 + c0 : 1 + c0 + w],
                        start=True,
                        stop=False,
                    )
                for c0, w in COL_CHUNKS:
                    nc.tensor.matmul(
                        psum[:, c0 : c0 + w],
                        wid_t[:p, :],
                        p2b[:p, c0 : c0 + w],
                        start=False,
                        stop=False,
                    )
                for c0, w in COL_CHUNKS:
                    nc.tensor.matmul(
                        psum[:, c0 : c0 + w],
                        wid_t[:p, :],
                        p2b[:p, 2 + c0 : 2 + c0 + w],
                        start=False,
                        stop=True,
                    )

                prod = pool.tile([128, INT], F32, tag="prod")
                nc.vector.tensor_mul(prod[:p, :], psum[:p, :], ac[:p, :])
                tt = pool.tile([128, INT], F32, tag="tt")
                nc.vector.scalar_tensor_tensor(
                    out=tt[:p, :],
                    in0=p2t[:p, 1 : 1 + INT],
                    scalar=2.0,
                    in1=p1t[:p, :],
                    op0=mybir.AluOpType.mult,
                    op1=mybir.AluOpType.subtract,
                )
                # final sum lands in vat (dead after ac) to save a slot
                nc.vector.tensor_add(vat[:p, :], tt[:p, :], prod[:p, :])
                nc.sync.dma_start(out=outs[r0 : r0 + p, :], in_=vat[:p, :])

            # ── column-stacked tail: 12 rows x 2112 cols as TG blocks of
            # [12(+2) rows, 264(+2) cols] so the per-op free-dim cost is
            # ~264 instead of 2112 ──
            kp, op_ = 14 * TG, TROWS * TG  # 112 input / 96 output parts
            tc2 = TCW + 2
            p2tt = tpool.tile([kp, tc2], F32, tag="t_p2")
            nc.sync.dma_start(out=p2tt[:], in_=p2tl[:])
            p2ct = tpool.tile([op_, tc2], F32, tag="t_p2c")
            nc.sync.dma_start(out=p2ct[:], in_=p2ctl[:])
            p1tt = tpool.tile([op_, TCW], F32, tag="t_p1")
            nc.scalar.dma_start(out=p1tt[:], in_=p1tl[:])
            vatt = tpool.tile([op_, TCW], F32, tag="t_va")
            nc.scalar.dma_start(out=vatt[:], in_=vatl[:])

            p2bt = tpool.tile([kp, tc2], BF16, tag="t_p2b")
            nc.scalar.copy(out=p2bt[:], in_=p2tt[:])
            act = tpool.tile([op_, TCW], F32, tag="t_ac")
            nc.scalar.activation(
                out=act[:],
                in_=vatt[:],
                func=mybir.ActivationFunctionType.Square,
                scale=float(DT / DX),
            )
            psum_t = ppool.tile([128, TCW], F32, tag="t_psum")
            nc.tensor.matmul(
                psum_t[:op_, :], wtb_t[:kp, :op_], p2bt[:, 1 : 1 + TCW],
                start=True, stop=False,
            )
            nc.tensor.matmul(
                psum_t[:op_, :], wti_t[:kp, :op_], p2bt[:, 0:TCW],
                start=False, stop=False,
            )
            nc.tensor.matmul(
                psum_t[:op_, :], wti_t[:kp, :op_], p2bt[:, 2 : 2 + TCW],
                start=False, stop=True,
            )
            prod_t = tpool.tile([op_, TCW], F32, tag="t_prod")
            nc.vector.tensor_mul(prod_t[:], psum_t[:op_, :], act[:])
            tt_t = tpool.tile([op_, TCW], F32, tag="t_tt")
            nc.vector.scalar_tensor_tensor(
                out=tt_t[:],
                in0=p2ct[:, 1 : 1 + TCW],
                scalar=2.0,
                in1=p1tt[:],
                op0=mybir.AluOpType.mult,
                op1=mybir.AluOpType.subtract,
            )
            nc.vector.tensor_add(vatt[:], tt_t[:], prod_t[:])
            nc.sync.dma_start(out=outs_t[:], in_=vatt[:])
    return nc


_ENGINES = {"SP", "PE", "DVE", "Activation", "Pool"}


def _split_waits_json(raw, maxw=1):
    """The walrus build here rejects instructions carrying more than a
    couple of semaphore waits. Engine streams execute in order, so any
    excess waits can be hoisted onto same-engine NoOps emitted directly
    before the instruction — semantically identical, codegen-legal."""
    m = json.loads(raw)
    n = 0
    for f in m["functions"]:
        for blk in f["blocks"]:
            out = []
            for inst in blk["instructions"]:
                si = inst.get("sync_info")
                waits = (si or {}).get("on_wait") or []
                if len(waits) > maxw and inst.get("engine") in _ENGINES:
                    for w in waits[:-maxw]:
                        n += 1
                        out.append(
                            {
                                "name": f"I-splitw{n}",
                                "opcode": "NoOp",
                                "engine": inst["engine"],
                                "ins": [],
                                "outs": [],
                                "sync_info": {"on_update": [], "on_wait": [w]},
                            }
                        )
                    si["on_wait"] = waits[-maxw:]
                out.append(inst)
            blk["instructions"] = out
    return json.dumps(m).encode()


_CACHE = {}


def _get_nc():
    if "nc" not in _CACHE:
        nc = _build_nc()
        orig = nc.to_json_bytes
        nc.to_json_bytes = lambda: _split_waits_json(orig())
        _CACHE["nc"] = nc
    return _CACHE["nc"]


def _stack_cols(a, w):
    """[r, TG*w(+overlap)] -> [TG*r, w] taking per-block column windows
    of width w at stride TCW."""
    r = a.shape[0]
    out = np.empty((TG * r, w), np.float32)
    for g in range(TG):
        out[g * r : (g + 1) * r, :] = a[:, g * TCW : g * TCW + w]
    return out


def _shard_inputs(p1, p2, varray):
    p1 = np.asarray(p1, np.float32)
    p2 = np.asarray(p2, np.float32)
    varray = np.asarray(varray, np.float32)
    wlap, wid, wtb, wti = _CACHE.setdefault("weights", _build_weights())
    ci = np.clip(np.arange(8, 8 + INT) - PML, 0, varray.shape[1] - 1)
    in_maps = []
    body = ROWS - TROWS  # 252 rows in the two full tiles
    for c in range(NCORES):
        g0 = 8 + ROWS * c
        ri = np.clip(np.arange(g0, g0 + ROWS) - PML, 0, varray.shape[0] - 1)
        p1_sh = p1[g0 : g0 + ROWS, 8 : 8 + INT]
        p2_sh = p2[g0 - 1 : g0 + ROWS + 1, 7 : 7 + PCOLS]
        va_sh = varray[np.ix_(ri, ci)]
        in_maps.append(
            {
                "p1s": np.ascontiguousarray(p1_sh[:body]),
                "p2s": np.ascontiguousarray(p2_sh),
                "vas": np.ascontiguousarray(va_sh[:body]),
                "wlap": wlap,
                "wid": wid,
                "wtb": wtb,
                "wti": wti,
                "p2tl": _stack_cols(p2_sh[TR0 : TR0 + TROWS + 2], TCW + 2),
                "p2ctl": _stack_cols(p2_sh[TR0 + 1 : TR0 + 1 + TROWS], TCW + 2),
                "p1tl": _stack_cols(p1_sh[TR0:], TCW),
                "vatl": _stack_cols(va_sh[TR0:], TCW),
            }
        )
    return in_maps


def run(p1, p2, varray, source_function, x_s, y_s, t, trace=False):
    """Run the device kernel; returns ((p, col), BassKernelResults)."""
    if trace:
        _install_ntff_hook()
    in_maps = _shard_inputs(p1, p2, varray)
    res = run_bass_kernel_spmd(
        _get_nc(), in_maps, core_ids=list(range(NCORES)), trace=trace
    )
    p = np.zeros((NP, NP), np.float32)
    body = ROWS - TROWS
    for c in range(NCORES):
        g0 = 8 + ROWS * c
        p[g0 : g0 + body, 8 : 8 + INT] = res.results[c]["outs"]
        ot = res.results[c]["outs_t"].reshape(TG, TROWS, TCW)
        tail = np.concatenate(list(ot), axis=1)  # [TROWS, INT]
        p[g0 + TR0 : g0 + ROWS, 8 : 8 + INT] = tail
    sf = np.asarray(source_function)
    p[int(x_s) + PML, int(y_s) + PML] += np.float32(float(sf[int(t)]) * DT**2)
    col = np.ascontiguousarray(p[PML:-PML, PML:-PML][:, 50])
    return (p, col), res


def kernel(p1, p2, varray, source_function, x_s, y_s, t):
    (p, col), _ = run(p1, p2, varray, source_function, x_s, y_s, t)
    return (p, col)


# revision 15
# speedup vs baseline: 1.4536x; 1.1209x over previous
"""Trainium2 Bass kernel for the single-step 2D wave-equation stencil
(nn_CustomRNN_12987981103489).

The reference computes, for the 2112x2112 interior of a 2128x2128 grid:

    out = 2*p2 - p1 + alpha * ((d2p/dx2) + (d2p/dy2))
    alpha = edge_pad(varray, 40)^2 * DT^2,  second differences / DX^2

plus a point-source add and an 8-wide zero frame. The Laplacian term is
scaled by varray^2 * DT^2 / DX^2 ~ 2.5e-9, i.e. it contributes ~1e-8
relative to the dominant f32 term 2*p2 - p1, so it is computed in bf16
on the TensorEngine (tridiagonal matmuls handle the partition-axis
stencil shifts, which the per-lane DVE cannot do), while the dominant
term stays in f32 on the VectorEngine. Measured vs the reference this
lands at ~2e-11 norm relative error.

Sharding: 1D row partition of the 2112 interior rows across 8 cores
(264 rows each); each core's p2 slice carries a 1-row halo on each side
so no inter-core communication is needed for this single step.

Layout trick: compute-engine APs must start at partition 0/32/64/96, so
each row tile keeps its 126 center rows at partitions 0..125 and stashes
the north/south halo rows at partitions 126/127; the halo contribution
to the vertical stencil is folded into the weight matrix columns.
"""

import json

import numpy as np
import ml_dtypes

import concourse.bass as bass
import concourse.mybir as mybir
from concourse.tile import TileContext
from concourse.vector_clock import ScopedClock
from concourse.bass_utils import run_bass_kernel_spmd

PML = 40
DT = 0.0005
DX = 10.0
NP = 2128          # padded grid size
INT = 2112         # interior size (8-wide frame)
NCORES = 8
ROWS = INT // NCORES   # 264 output rows per core
PCOLS = INT + 2        # 2114 p2 columns per core (1-col halo each side)
# per-core full row tiles: (row offset within shard, output rows); the
# 12-row remainder is handled separately in a column-stacked layout
TILES = [(0, 126), (126, 126)]
COL_CHUNKS = [(0, 512), (512, 512), (1024, 512), (1536, 512), (2048, 64)]
# tail: rows 252..263 restacked as 8 column-chunks of 264 cols
TR0 = 252          # first tail row (shard coords)
TROWS = 12
TG = 8             # column chunks
TCW = INT // TG    # 264 columns per chunk
F32 = mybir.dt.float32
BF16 = mybir.dt.bfloat16


def _install_ntff_hook():
    """Best-effort registration of the axon NTFF profile hook so
    trace=True (or BASS_TRACE=1) yields a real neuron-profile exec time.
    The image's antenv package lacks axon_hooks; provide the module shim
    and wire the ctypes hook from trn_agent_boot. Harmless if absent."""
    import sys as _sys
    import types as _types

    if "antenv.axon_hooks" in _sys.modules:
        return
    try:
        mod = _types.ModuleType("antenv.axon_hooks")
        state = {"hook": None}
        mod.set_axon_ntff_profile_hook = lambda h: state.__setitem__("hook", h)
        mod.get_axon_ntff_profile_hook = lambda: state["hook"]
        from trn_agent_boot.trn_boot import _ntff_profile_via_ctypes

        hook = _ntff_profile_via_ctypes("/opt/axon/libaxon_pjrt.so")
        mod.set_axon_ntff_profile_hook(hook)
        _sys.modules["antenv.axon_hooks"] = mod
    except Exception:
        pass


class _SplitDrainTileContext(TileContext):
    """The walrus build in this container rejects >2 sync waits on one
    TPB_CTRL instruction; the stock TileContext funnels the whole global
    clock into a single kernel-tail Drain. Re-emit those waits one per
    SP NOP instead."""

    def _drain_and_barrier(self, tick_clock, wait_clock):
        drain_inst = self.nc.sync.drain()
        wait_clock.add_sem_waits(
            drain_inst.ins, ScopedClock({None: tick_clock.global_clock})
        )
        si = drain_inst.ins.sync_info
        waits = list(si.on_wait) if si is not None else []
        if len(waits) > 1:
            drain_inst.ins.sync_info = mybir.SyncInfo(
                on_update=list(si.on_update), on_wait=[]
            )
            for w in waits:
                nop = self.nc.sync.nop(nofuse=True, hint="split_drain_wait")
                nop.ins.sync_info = mybir.SyncInfo(on_update=[], on_wait=[w])

        self.nc.all_engine_barrier()
        assert self.sems is not None
        popped = self.nc._tile_sem_poison_stack.pop()
        assert popped is self._sem_poison
        self.nc.clear_and_free_semaphores(list(self.sems.allocated().values()))
        self.nc.all_engine_barrier()


def _lap_weights(p):
    """Vertical-stencil weight matrix for a tile with p center rows at
    partitions 0..p-1, north halo row at partition p, south halo row at
    partition p+1 (lhsT layout: out = lhsT.T @ rhs, contraction over
    partitions). Out row j = -4*C[j] + C[j-1] + C[j+1] (+halo at the
    edges); columns >= p are zero so the unused psum rows read 0."""
    w = np.zeros((128, 128), np.float32)
    for j in range(p):
        w[j, j] = -4.0
        if j > 0:
            w[j - 1, j] = 1.0
        if j < p - 1:
            w[j + 1, j] = 1.0
    w[p, 0] = 1.0        # north halo feeds out row 0
    w[p + 1, p - 1] = 1.0  # south halo feeds out row p-1
    return w.astype(ml_dtypes.bfloat16)


def _tail_weights():
    """Block weights for the column-stacked 12-row tail: TG blocks, each
    14 input partitions (12 rows + both halos, natural order) and 12
    output partitions."""
    wtb = np.zeros((128, 128), np.float32)
    wti = np.zeros((128, 128), np.float32)
    for g in range(TG):
        bk, bj = 14 * g, TROWS * g
        for j in range(TROWS):
            wtb[bk + j, bj + j] = 1.0       # north
            wtb[bk + j + 1, bj + j] = -4.0  # center
            wtb[bk + j + 2, bj + j] = 1.0   # south
            wti[bk + j + 1, bj + j] = 1.0   # center identity (E/W)
    return wtb.astype(ml_dtypes.bfloat16), wti.astype(ml_dtypes.bfloat16)


def _build_weights():
    wlap = _lap_weights(126)
    wid = np.eye(128, dtype=np.float32).astype(ml_dtypes.bfloat16)
    wtb, wti = _tail_weights()
    return wlap, wid, wtb, wti


def _build_nc():
    nc = bass.Bass()
    p1s = nc.dram_tensor("p1s", [ROWS - TROWS, INT], F32, kind="ExternalInput")
    p2s = nc.dram_tensor("p2s", [ROWS + 2, PCOLS], F32, kind="ExternalInput")
    vas = nc.dram_tensor("vas", [ROWS - TROWS, INT], F32, kind="ExternalInput")
    wlap = nc.dram_tensor("wlap", [128, 128], BF16, kind="ExternalInput")
    wid = nc.dram_tensor("wid", [128, 128], BF16, kind="ExternalInput")
    wtb = nc.dram_tensor("wtb", [128, 128], BF16, kind="ExternalInput")
    wti = nc.dram_tensor("wti", [128, 128], BF16, kind="ExternalInput")
    # column-stacked 12-row tail (prepared host-side): TG blocks of
    # (12+2 halo) p2 rows / 12 rows for p1+varray, 264(+2) cols each
    p2tl = nc.dram_tensor("p2tl", [14 * TG, TCW + 2], F32, kind="ExternalInput")
    p2ctl = nc.dram_tensor("p2ctl", [TROWS * TG, TCW + 2], F32, kind="ExternalInput")
    p1tl = nc.dram_tensor("p1tl", [TROWS * TG, TCW], F32, kind="ExternalInput")
    vatl = nc.dram_tensor("vatl", [TROWS * TG, TCW], F32, kind="ExternalInput")
    outs = nc.dram_tensor("outs", [ROWS - TROWS, INT], F32, kind="ExternalOutput")
    outs_t = nc.dram_tensor(
        "outs_t", [TROWS * TG, TCW], F32, kind="ExternalOutput"
    )

    with _SplitDrainTileContext(nc) as tc:
        with (
            tc.tile_pool(name="wpool", bufs=1) as wpool,
            tc.tile_pool(name="sbuf", bufs=2) as pool,
            tc.tile_pool(name="tpool", bufs=1) as tpool,
            tc.tile_pool(name="psum", bufs=1, space="PSUM") as ppool,
        ):
            # ── phase 1: all input DMAs up front, on SP only, in
            # priority order (tile0, weights, tile1, tail). The HWDGE
            # queues drain FIFO, so this ordering controls which data
            # lands first; issuing from one otherwise-idle sequencer
            # keeps issue off the compute engines' critical path. ──
            body = []
            for r0, p in TILES:
                vat = pool.tile([128, INT], F32, tag="vat")
                p2t = pool.tile([128, PCOLS], F32, tag="p2t")
                p1t = pool.tile([128, INT], F32, tag="p1t")
                nc.sync.dma_start(out=p2t[:p, :], in_=p2s[r0 + 1 : r0 + 1 + p, :])
                nc.sync.dma_start(
                    out=p2t[p : p + 2, :], in_=p2s[r0 : r0 + p + 2 : p + 1, :]
                )
                nc.sync.dma_start(out=vat[:p, :], in_=vas[r0 : r0 + p, :])
                nc.sync.dma_start(out=p1t[:p, :], in_=p1s[r0 : r0 + p, :])
                body.append((r0, p, vat, p2t, p1t))
                if r0 == 0:
                    wlap_t = wpool.tile([128, 128], BF16, tag="wlap")
                    nc.sync.dma_start(out=wlap_t[:], in_=wlap[:])
                    wid_t = wpool.tile([128, 128], BF16, tag="wid")
                    nc.sync.dma_start(out=wid_t[:], in_=wid[:])
                    wtb_t = wpool.tile([128, 128], BF16, tag="wtb")
                    nc.sync.dma_start(out=wtb_t[:], in_=wtb[:])
                    wti_t = wpool.tile([128, 128], BF16, tag="wti")
                    nc.sync.dma_start(out=wti_t[:], in_=wti[:])

            kp, op_ = 14 * TG, TROWS * TG  # 112 input / 96 output parts
            tc2 = TCW + 2
            p2tt = tpool.tile([kp, tc2], F32, tag="t_p2")
            nc.sync.dma_start(out=p2tt[:], in_=p2tl[:])
            p2ct = tpool.tile([op_, tc2], F32, tag="t_p2c")
            nc.sync.dma_start(out=p2ct[:], in_=p2ctl[:])
            p1tt = tpool.tile([op_, TCW], F32, tag="t_p1")
            nc.sync.dma_start(out=p1tt[:], in_=p1tl[:])
            vatt = tpool.tile([op_, TCW], F32, tag="t_va")
            nc.sync.dma_start(out=vatt[:], in_=vatl[:])

            # ── phase 2: compute (big tiles first, cheap tail last) ──
            for r0, p, vat, p2t, p1t in body:
                k = p + 2
                # bf16 copy of p2 for the TensorEngine stencil
                p2b = pool.tile([128, PCOLS], BF16, tag="p2b")
                nc.scalar.copy(out=p2b[:k, :], in_=p2t[:k, :])
                # alpha/dx^2 = (varray * (DT/DX))^2
                ac = pool.tile([128, INT], F32, tag="ac")
                nc.scalar.activation(
                    out=ac[:p, :],
                    in_=vat[:p, :],
                    func=mybir.ActivationFunctionType.Square,
                    scale=float(DT / DX),
                )
                # 5-point Laplacian accumulated in PSUM: vertical part
                # incl. halo via wlap, then +west, +east (grouped by
                # weight matrix to minimize LDWEIGHTS swaps)
                psum = ppool.tile([128, INT], F32, tag="psum")
                for c0, w in COL_CHUNKS:
                    nc.tensor.matmul(
                        psum[:, c0 : c0 + w],
                        wlap_t[:k, :],
                        p2b[:k, 1 + c0 : 1 + c0 + w],
                        start=True,
                        stop=False,
                    )
                for c0, w in COL_CHUNKS:
                    nc.tensor.matmul(
                        psum[:, c0 : c0 + w],
                        wid_t[:p, :],
                        p2b[:p, c0 : c0 + w],
                        start=False,
                        stop=False,
                    )
                for c0, w in COL_CHUNKS:
                    nc.tensor.matmul(
                        psum[:, c0 : c0 + w],
                        wid_t[:p, :],
                        p2b[:p, 2 + c0 : 2 + c0 + w],
                        start=False,
                        stop=True,
                    )
                prod = pool.tile([128, INT], F32, tag="prod")
                nc.vector.tensor_mul(prod[:p, :], psum[:p, :], ac[:p, :])
                tt = pool.tile([128, INT], F32, tag="tt")
                nc.vector.scalar_tensor_tensor(
                    out=tt[:p, :],
                    in0=p2t[:p, 1 : 1 + INT],
                    scalar=2.0,
                    in1=p1t[:p, :],
                    op0=mybir.AluOpType.mult,
                    op1=mybir.AluOpType.subtract,
                )
                # final sum lands in vat (dead after ac) to save a slot
                nc.vector.tensor_add(vat[:p, :], tt[:p, :], prod[:p, :])
                nc.scalar.dma_start(out=outs[r0 : r0 + p, :], in_=vat[:p, :])

            # tail compute: every op ~8x cheaper in free-dim length
            p2bt = tpool.tile([kp, tc2], BF16, tag="t_p2b")
            nc.scalar.copy(out=p2bt[:], in_=p2tt[:])
            act = tpool.tile([op_, TCW], F32, tag="t_ac")
            nc.scalar.activation(
                out=act[:],
                in_=vatt[:],
                func=mybir.ActivationFunctionType.Square,
                scale=float(DT / DX),
            )
            psum_t = ppool.tile([128, TCW], F32, tag="t_psum")
            nc.tensor.matmul(
                psum_t[:op_, :], wtb_t[:kp, :op_], p2bt[:, 1 : 1 + TCW],
                start=True, stop=False,
            )
            nc.tensor.matmul(
                psum_t[:op_, :], wti_t[:kp, :op_], p2bt[:, 0:TCW],
                start=False, stop=False,
            )
            nc.tensor.matmul(
                psum_t[:op_, :], wti_t[:kp, :op_], p2bt[:, 2 : 2 + TCW],
                start=False, stop=True,
            )
            prod_t = tpool.tile([op_, TCW], F32, tag="t_prod")
            nc.vector.tensor_mul(prod_t[:], psum_t[:op_, :], act[:])
            tt_t = tpool.tile([op_, TCW], F32, tag="t_tt")
            nc.vector.scalar_tensor_tensor(
                out=tt_t[:],
                in0=p2ct[:, 1 : 1 + TCW],
                scalar=2.0,
                in1=p1tt[:],
                op0=mybir.AluOpType.mult,
                op1=mybir.AluOpType.subtract,
            )
            nc.vector.tensor_add(vatt[:], tt_t[:], prod_t[:])
            nc.scalar.dma_start(out=outs_t[:], in_=vatt[:])
    return nc


_ENGINES = {"SP", "PE", "DVE", "Activation", "Pool"}


def _split_waits_json(raw, maxw=1):
    """The walrus build here rejects instructions carrying more than a
    couple of semaphore waits. Engine streams execute in order, so any
    excess waits can be hoisted onto same-engine NoOps emitted directly
    before the instruction — semantically identical, codegen-legal."""
    m = json.loads(raw)
    n = 0
    for f in m["functions"]:
        for blk in f["blocks"]:
            out = []
            for inst in blk["instructions"]:
                si = inst.get("sync_info")
                waits = (si or {}).get("on_wait") or []
                if len(waits) > maxw and inst.get("engine") in _ENGINES:
                    for w in waits[:-maxw]:
                        n += 1
                        out.append(
                            {
                                "name": f"I-splitw{n}",
                                "opcode": "NoOp",
                                "engine": inst["engine"],
                                "ins": [],
                                "outs": [],
                                "sync_info": {"on_update": [], "on_wait": [w]},
                            }
                        )
                    si["on_wait"] = waits[-maxw:]
                out.append(inst)
            blk["instructions"] = out
    return json.dumps(m).encode()


_CACHE = {}


def _get_nc():
    if "nc" not in _CACHE:
        nc = _build_nc()
        orig = nc.to_json_bytes
        nc.to_json_bytes = lambda: _split_waits_json(orig())
        _CACHE["nc"] = nc
    return _CACHE["nc"]


def _stack_cols(a, w):
    """[r, TG*w(+overlap)] -> [TG*r, w] taking per-block column windows
    of width w at stride TCW."""
    r = a.shape[0]
    out = np.empty((TG * r, w), np.float32)
    for g in range(TG):
        out[g * r : (g + 1) * r, :] = a[:, g * TCW : g * TCW + w]
    return out


def _shard_inputs(p1, p2, varray):
    p1 = np.asarray(p1, np.float32)
    p2 = np.asarray(p2, np.float32)
    varray = np.asarray(varray, np.float32)
    wlap, wid, wtb, wti = _CACHE.setdefault("weights", _build_weights())
    ci = np.clip(np.arange(8, 8 + INT) - PML, 0, varray.shape[1] - 1)
    in_maps = []
    body = ROWS - TROWS  # 252 rows in the two full tiles
    for c in range(NCORES):
        g0 = 8 + ROWS * c
        ri = np.clip(np.arange(g0, g0 + ROWS) - PML, 0, varray.shape[0] - 1)
        p1_sh = p1[g0 : g0 + ROWS, 8 : 8 + INT]
        p2_sh = p2[g0 - 1 : g0 + ROWS + 1, 7 : 7 + PCOLS]
        va_sh = varray[np.ix_(ri, ci)]
        in_maps.append(
            {
                "p1s": np.ascontiguousarray(p1_sh[:body]),
                "p2s": np.ascontiguousarray(p2_sh),
                "vas": np.ascontiguousarray(va_sh[:body]),
                "wlap": wlap,
                "wid": wid,
                "wtb": wtb,
                "wti": wti,
                "p2tl": _stack_cols(p2_sh[TR0 : TR0 + TROWS + 2], TCW + 2),
                "p2ctl": _stack_cols(p2_sh[TR0 + 1 : TR0 + 1 + TROWS], TCW + 2),
                "p1tl": _stack_cols(p1_sh[TR0:], TCW),
                "vatl": _stack_cols(va_sh[TR0:], TCW),
            }
        )
    return in_maps


def run(p1, p2, varray, source_function, x_s, y_s, t, trace=False):
    """Run the device kernel; returns ((p, col), BassKernelResults)."""
    if trace:
        _install_ntff_hook()
    in_maps = _shard_inputs(p1, p2, varray)
    res = run_bass_kernel_spmd(
        _get_nc(), in_maps, core_ids=list(range(NCORES)), trace=trace
    )
    p = np.zeros((NP, NP), np.float32)
    body = ROWS - TROWS
    for c in range(NCORES):
        g0 = 8 + ROWS * c
        p[g0 : g0 + body, 8 : 8 + INT] = res.results[c]["outs"]
        ot = res.results[c]["outs_t"].reshape(TG, TROWS, TCW)
        tail = np.concatenate(list(ot), axis=1)  # [TROWS, INT]
        p[g0 + TR0 : g0 + ROWS, 8 : 8 + INT] = tail
    sf = np.asarray(source_function)
    p[int(x_s) + PML, int(y_s) + PML] += np.float32(float(sf[int(t)]) * DT**2)
    col = np.ascontiguousarray(p[PML:-PML, PML:-PML][:, 50])
    return (p, col), res


def kernel(p1, p2, varray, source_function, x_s, y_s, t):
    (p, col), _ = run(p1, p2, varray, source_function, x_s, y_s, t)
    return (p, col)
